# revision 28
# baseline (speedup 1.0000x reference)
"""EquiConv (DeepH-E3) Trainium2 kernel — 8-core data-parallel over edges.

v2 strategy (bf16 matmul pipeline, host-fused prescales):
  - The per-edge x2 scalar multiplies commute with every channel-space
    matmul, so the host pre-scales the channel-major activation blocks
    (x1s*s, x1s*v_i, x1v_i*s, x1v_i*v_i) and ships ONE packed bf16
    tensor [128, 8, E] per core.  The device is then a pure bf16 matmul
    pipeline: 15 accumulating matmuls per 512-edge tile, 4 ACT
    activations, 4 DVE elementwise ops, one load DMA + two store DMAs.
  - FC biases are folded into the fc2 matmuls via a ones-row (K=65);
    the gate/wwb columns are duplicated to 128 partitions so the
    vec0/vec1 gating is a single [128]-row op; the two p3 (1o x 0e)
    matmuls for components 0/1 are merged into one block-diagonal
    matmul.
  - Outputs are stored as bf16 [128, 3, E] and upcast/transposed on the
    host.

Self-contained: hardcodes shapes from the problem spec; no file reads.
"""
import os
import sys

import numpy as np

# ---------------------------------------------------------------- constants
E_FULL = 200000
N_CORES = 8
E_CORE = E_FULL // N_CORES      # 25000
NT = 512                        # edges per tile
T_TILES = 49                    # tiles per core
E_PAD = NT * T_TILES            # 25088
MUL_S = 128
MUL_V = 64

INV_S = 1.0 / np.sqrt(MUL_S)
INV_V = 1.0 / np.sqrt(MUL_V)
SQ2 = 1.0 / np.sqrt(2.0)
SQ3 = 1.0 / np.sqrt(3.0)

_REPO_CANDIDATES = (
    "/opt/trn_rl_repo",
    "/root/.axon_site/_ro/trn_rl_repo",
)


def _ensure_repo_on_path():
    try:
        import concourse.bass  # noqa: F401
        return
    except ImportError:
        pass
    for p in _REPO_CANDIDATES:
        if os.path.isdir(p) and p not in sys.path:
            sys.path.insert(0, p)
    import concourse.bass  # noqa: F401


_CACHE = {}


def _build_nc():
    """Build + compile the per-core Bass program (cached)."""
    if "nc" in _CACHE:
        return _CACHE["nc"]
    _ensure_repo_on_path()
    import concourse.mybir as mybir
    import concourse.tile as tile
    from concourse import bacc

    F32 = mybir.dt.float32
    BF16 = mybir.dt.bfloat16
    MULT = mybir.AluOpType.mult
    ADD = mybir.AluOpType.add
    AF = mybir.ActivationFunctionType

    nc = bacc.Bacc(trn_type="TRN2", target_bir_lowering=False, debug=False,
                   num_devices=N_CORES)

    # DRAM IO (per-core shard, channel-major, bf16) ------------------------
    # in_pack blocks: 0 x1s*s | 1 x1s*v0 | 2 x1s*v1 | 3 x1s*v2
    #                 4 xv_s01 | 5 xv_p01 | 6 [x1v2*v2 ; -] | 7 fw
    #                 8 [h1s-space ; x1v2*s]  (merged-matmul rhs)
    d_in = nc.dram_tensor("in_pack", [128, 9, E_PAD], BF16,
                          kind="ExternalInput")
    # folded weights ([K, M] layouts, ready as lhsT)
    d_wa0 = nc.dram_tensor("wa0", [128, 128], BF16, kind="ExternalInput")
    d_wa1d = nc.dram_tensor("wa1d", [128, 128], BF16, kind="ExternalInput")
    d_wp2 = nc.dram_tensor("wp2", [128, 64], BF16, kind="ExternalInput")
    d_wbd = nc.dram_tensor("wbd", [128, 128], BF16, kind="ExternalInput")
    d_wb4s = nc.dram_tensor("wb4s", [128, 128], BF16, kind="ExternalInput")
    d_wb5sd = nc.dram_tensor("wb5sd", [128, 128], BF16, kind="ExternalInput")
    d_wb4b = nc.dram_tensor("wb4b", [64, 128], BF16, kind="ExternalInput")
    d_wb5bd = nc.dram_tensor("wb5bd", [64, 128], BF16, kind="ExternalInput")
    d_fc0 = nc.dram_tensor("fc0", [128, 64], BF16, kind="ExternalInput")
    # merged block-diag lhsT: [[fc1, 0], [0, wc2]] -> out [h2 ; vec2_p3]
    d_wmrg = nc.dram_tensor("wmrg", [128, 128], BF16, kind="ExternalInput")
    d_fc2a = nc.dram_tensor("fc2a", [65, 128], BF16, kind="ExternalInput")
    d_fc2b = nc.dram_tensor("fc2b", [65, 128], BF16, kind="ExternalInput")
    d_b0 = nc.dram_tensor("b0c", [64, 1], F32, kind="ExternalInput")
    d_b1 = nc.dram_tensor("b1c", [64, 1], F32, kind="ExternalInput")

    # out blocks: 0 scal' | 1 vec01' | 2 vec2' (rows 0:64)
    d_out = nc.dram_tensor("out_t", [128, 3, E_PAD], BF16,
                           kind="ExternalOutput")

    with tile.TileContext(nc) as tc:
        with tc.tile_pool(name="const", bufs=1) as cp, \
             tc.tile_pool(name="io", bufs=5) as io, \
             tc.tile_pool(name="work", bufs=2) as wk, \
             tc.tile_pool(name="ps", bufs=1, space="PSUM") as ps:

            # constants into SBUF once
            def const(d, shape, dtype=BF16, name=None):
                t = cp.tile(shape, dtype, name=name or d.name + "_sb")
                nc.sync.dma_start(t, d.ap())
                return t

            w_wa0 = const(d_wa0, [128, 128])
            w_wa1d = const(d_wa1d, [128, 128])
            w_wp2 = const(d_wp2, [128, 64])
            w_wbd = const(d_wbd, [128, 128])
            w_wb4s = const(d_wb4s, [128, 128])
            w_wb5sd = const(d_wb5sd, [128, 128])
            w_wb4b = const(d_wb4b, [64, 128])
            w_wb5bd = const(d_wb5bd, [64, 128])
            w_fc0 = const(d_fc0, [128, 64])
            w_mrg = const(d_wmrg, [128, 128])
            w_fc2a = const(d_fc2a, [65, 128])
            w_fc2b = const(d_fc2b, [65, 128])
            c_b0 = const(d_b0, [64, 1], F32)
            c_b1 = const(d_b1, [64, 1], F32)

            # persistent h2s buffers with a ones-row (bias fold, K=65)
            h2s_bufs = []
            for i in range(2):
                t = cp.tile([65, NT], BF16, name=f"h2s_{i}")
                nc.gpsimd.memset(t[64:65, :], 1.0)
                h2s_bufs.append(t)

            for t in range(T_TILES):
                sl = slice(t * NT, (t + 1) * NT)

                # ---- load (one DMA on the Pool queue) ----------------
                xin = io.tile([128, 9, NT], BF16)
                nc.gpsimd.dma_start(xin, d_in.ap()[:, :, sl])
                x_ss = xin[:, 0, :]
                x_sv0 = xin[:, 1, :]
                x_sv1 = xin[:, 2, :]
                x_sv2 = xin[:, 3, :]
                x_v_s01 = xin[:, 4, :]
                x_v_p01 = xin[:, 5, :]
                x_p2 = xin[0:64, 6, :]
                x_fw = xin[:, 7, :]
                # merged-matmul rhs: rows 0:64 get h1s from ACT below,
                # rows 64:128 hold x1v2*s straight from the load
                mix = xin[:, 8, :]

                # ---- PE stream: ordered so (a) the PE never waits on
                # an activation round-trip, (b) each PSUM bank's t+1
                # writer trails its t consumer with slack ---------------
                # h1 borrows m2v's bank rows 0:64 (dead once h1s is read;
                # the merged matmul overwrites it with start=True)
                m2v = ps.tile([128, NT], F32, tag="m2v", bufs=2)
                h1 = m2v[0:64, :]
                nc.tensor.matmul(h1, w_fc0, x_fw, start=True, stop=True,
                                 skip_group_check=True)
                nc.scalar.activation(xin[0:64, 8, :], h1, AF.Silu,
                                     bias=c_b0)

                scal = ps.tile([128, NT], F32, tag="scal")
                gate = ps.tile([128, NT], F32, tag="gate")
                nc.tensor.matmul(scal, w_wa0, x_ss, start=True, stop=False)
                nc.tensor.matmul(gate, w_wa1d, x_ss, start=True, stop=False)
                nc.tensor.matmul(scal, w_wb4s, x_v_p01,
                                 start=False, stop=False)
                nc.tensor.matmul(gate, w_wb5sd, x_v_p01,
                                 start=False, stop=False)
                # merged: rows 0:64 = h2 (fc1 @ h1s), 64:128 = wc2 @ xs2
                # (early so the h2s round-trip lands before fc2a/fc2b)
                nc.tensor.matmul(m2v, w_mrg, mix, start=True, stop=True,
                                 skip_group_check=True)
                h2s = h2s_bufs[t % 2]
                nc.scalar.activation(h2s[0:64, :], m2v[0:64, :], AF.Silu,
                                     bias=c_b1)
                nc.tensor.matmul(scal, w_wb4b, x_p2,
                                 start=False, stop=True)
                nc.tensor.matmul(gate, w_wb5bd, x_p2,
                                 start=False, stop=True)

                # vec paths fill the PE while the h2s activation lands
                vec01 = ps.tile([128, NT], F32, tag="vec01", bufs=2)
                nc.tensor.matmul(vec01, w_wbd, x_v_s01,
                                 start=True, stop=False)
                nc.tensor.matmul(vec01[0:64, :], w_wp2, x_sv0,
                                 start=False, stop=True,
                                 skip_group_check=True)
                nc.tensor.matmul(vec01[64:128, :], w_wp2, x_sv1,
                                 start=False, stop=True,
                                 skip_group_check=True)

                sc_silu = wk.tile([128, NT], BF16)
                nc.scalar.activation(sc_silu, scal, AF.Silu)
                tgate = wk.tile([128, NT], BF16)
                nc.scalar.activation(tgate, gate, AF.Tanh, scale=0.5)

                # per-edge weights (biases folded via the ones-row)
                wwa = ps.tile([128, NT], F32, tag="wwa")
                nc.tensor.matmul(wwa, w_fc2a, h2s, start=True, stop=True)
                wwb = ps.tile([128, NT], F32, tag="wwb")
                nc.tensor.matmul(wwb, w_fc2b, h2s, start=True, stop=True)
                nc.tensor.matmul(m2v[64:128, :], w_wp2, x_sv2,
                                 start=False, stop=True,
                                 skip_group_check=True)

                # ---- gate + e3ElementWise ----------------------------
                # sgw = (tanh(g/2)+1) * 0.5*(w_vec+b) = sigmoid(g)*w_vec
                sgw = wk.tile([128, NT], BF16)
                nc.vector.scalar_tensor_tensor(sgw, tgate, 1.0, wwb,
                                               ADD, MULT)
                outp = wk.tile([128, 3, NT], BF16, bufs=4)
                nc.vector.tensor_tensor(outp[:, 1, :], vec01, sgw, MULT)
                nc.vector.tensor_tensor(outp[:, 0, :], sc_silu, wwa, MULT)
                # full-width: rows 0:64 are h2*sgw junk the host ignores
                nc.vector.tensor_tensor(outp[:, 2, :], m2v, sgw, MULT)

                # ---- store (sync queue: keeps the load queue free) ---
                nc.sync.dma_start(d_out.ap()[:, :, sl], outp)

    nc.compile()
    _CACHE["nc"] = nc
    return nc


def _fold_weights(inp):
    """Fold per-channel weights + constants into bf16 matmul matrices."""
    import ml_dtypes
    bf16 = ml_dtypes.bfloat16
    f = lambda k: np.asarray(inp[k], dtype=np.float32)
    w0f = f("w1_p0") * f("w2_p0")[None, :] * (INV_S * SQ2)
    w1f = f("w1_p1") * f("w2_p1")[None, :] * (INV_S * SQ2)
    w2f = f("w1_p2") * f("w2_p2")[None, :] * (INV_S * SQ2)
    w3f = f("w1_p3") * f("w2_p3")[None, :] * (INV_V * SQ2)
    w4f = f("w1_p4") * f("w2_p4")[None, :] * (INV_V * SQ3 * SQ2)
    w5f = f("w1_p5") * f("w2_p5")[None, :] * (INV_V * SQ3 * SQ2)
    fc2 = f("fc_w2")
    b2 = f("fc_b2")
    # block-diagonal p3 weight for components 0/1
    wbd = np.zeros((128, 128), np.float32)
    wbd[:64, :64] = w3f
    wbd[64:, 64:] = w3f
    # merged fc1 + wc2 block-diagonal: out = [h2 ; vec2_p3]
    wmrg = np.zeros((128, 128), np.float32)
    wmrg[:64, :64] = f("fc_w1")
    wmrg[64:, 64:] = w3f
    # fc2 with bias row (K=65); gate half duplicated to M=128, x0.5 folded
    fc2a = np.concatenate([fc2[:, :128], b2[None, :128]], axis=0)
    fc2b_h = np.concatenate([fc2[:, 128:], b2[None, 128:]], axis=0) * 0.5
    fc2b = np.concatenate([fc2b_h, fc2b_h], axis=1)
    dup_m = lambda w: np.concatenate([w, w], axis=1)
    c = lambda a: np.ascontiguousarray(a.astype(bf16))
    return {
        "wa0": c(w0f),
        "wa1d": c(dup_m(w1f)),
        "wp2": c(w2f),
        "wbd": c(wbd),
        "wb4s": c(np.concatenate([w4f, w4f], axis=0)),
        "wb5sd": c(dup_m(np.concatenate([w5f, w5f], axis=0))),
        "wb4b": c(w4f),
        "wb5bd": c(dup_m(w5f)),
        "fc0": c(f("fc_w0")),
        "wmrg": c(wmrg),
        "fc2a": c(fc2a),
        "fc2b": c(fc2b),
        "b0c": np.ascontiguousarray(f("fc_b0")[:, None]),
        "b1c": np.ascontiguousarray(f("fc_b1")[:, None]),
    }


def _shard_inputs(inp):
    """Per-core channel-major prescaled bf16 shards [128, 8, E_PAD]."""
    import ml_dtypes
    bf16 = ml_dtypes.bfloat16
    fea_in1 = np.asarray(inp["fea_in1"], dtype=np.float32)
    fea_in2 = np.asarray(inp["fea_in2"], dtype=np.float32)
    fea_w = np.asarray(inp["fea_weight"], dtype=np.float32)
    shards = []
    for ci in range(N_CORES):
        s = slice(ci * E_CORE, (ci + 1) * E_CORE)
        x1 = fea_in1[s]
        x2 = fea_in2[s]
        fw = fea_w[s]
        x1sT = x1[:, :128].T                                  # [128, E]
        x1v = x1[:, 128:].reshape(E_CORE, 64, 3)
        xv0 = x1v[:, :, 0].T                                  # [64, E]
        xv1 = x1v[:, :, 1].T
        xv2 = x1v[:, :, 2].T
        sc = x2[:, 0][None, :]
        v0 = x2[:, 1][None, :]
        v1 = x2[:, 2][None, :]
        v2 = x2[:, 3][None, :]
        pack = np.zeros((128, 9, E_PAD), bf16)
        pack[:, 0, :E_CORE] = x1sT * sc
        pack[:, 1, :E_CORE] = x1sT * v0
        pack[:, 2, :E_CORE] = x1sT * v1
        pack[:, 3, :E_CORE] = x1sT * v2
        pack[:64, 4, :E_CORE] = xv0 * sc
        pack[64:, 4, :E_CORE] = xv1 * sc
        pack[:64, 5, :E_CORE] = xv0 * v0
        pack[64:, 5, :E_CORE] = xv1 * v1
        pack[:64, 6, :E_CORE] = xv2 * v2
        pack[:, 7, :E_CORE] = fw.T
        pack[64:, 8, :E_CORE] = xv2 * sc
        shards.append({"in_pack": pack})
    return shards


def run(inputs, trace=False, trace_kwargs=None):
    """Run the kernel; returns (output [E,320] f32, BassKernelResults)."""
    _ensure_repo_on_path()
    from concourse import bass_utils

    nc = _build_nc()
    weights = _fold_weights(inputs)
    shards = _shard_inputs(inputs)
    in_maps = [{**weights, **sh} for sh in shards]

    kwargs = {}
    if trace:
        _install_ntff_hook()
        kwargs.update(trace=True, **(trace_kwargs or {}))
    res = bass_utils.run_bass_kernel_spmd(
        nc, in_maps, core_ids=list(range(N_CORES)), **kwargs)

    out = np.empty((E_FULL, 320), np.float32)
    for ci in range(N_CORES):
        o = np.asarray(res.results[ci]["out_t"])[:, :, :E_CORE]
        o = o.astype(np.float32)                        # [128, 3, E_CORE]
        s = slice(ci * E_CORE, (ci + 1) * E_CORE)
        out[s, :128] = o[:, 0, :].T
        vec = np.empty((E_CORE, 64, 3), np.float32)
        vec[:, :, 0] = o[0:64, 1, :].T
        vec[:, :, 1] = o[64:128, 1, :].T
        vec[:, :, 2] = o[64:128, 2, :].T
        out[s, 128:] = vec.reshape(E_CORE, 192)
    return out, res


def _install_ntff_hook():
    """Shim the missing antenv.axon_hooks so trace=True works under axon."""
    import types
    import antenv
    from concourse import bass_utils
    if "antenv.axon_hooks" in sys.modules:
        return
    mod = types.ModuleType("antenv.axon_hooks")
    _h = [None]
    mod.set_axon_ntff_profile_hook = lambda h: _h.__setitem__(0, h)
    mod.get_axon_ntff_profile_hook = lambda: _h[0]
    sys.modules["antenv.axon_hooks"] = mod
    antenv.axon_hooks = mod
    from trn_agent_boot.trn_boot import _ntff_profile_via_ctypes
    mod.set_axon_ntff_profile_hook(
        _ntff_profile_via_ctypes("/opt/axon/libaxon_pjrt.so"))
    bass_utils.upload_artifacts = lambda tmpdir: tmpdir


def kernel(**inputs) -> np.ndarray:
    out, _ = run(inputs, trace=False)
    return out
